# revision 69
# baseline (speedup 1.0000x reference)
"""Trainium2 Bass kernel for nn_MultiHeadDilatedState (B=4, S=4096, H=768).

Sharding: 8 cores = (batch b in 0..4) x (head-group g in 0..2); each core
runs the head phase (gate matmul + SwiGLU + dilated causal convs + neural
memory + router weighting) for its 6 heads over the full sequence in
feature-major layout.  The head->mix exchange is pipelined INTO the main
loop: core j owns the 64-token stripe [512c+64j, 512c+64j+64) of every
chunk c, so each chunk's finished slab is AllToAll'd as its own piece
(issued at iteration c+1) and mixed two iterations later while the loop
still runs.  Only the last chunk's piece + mix trail the loop (~35us tail
instead of a ~175us serial collective+mix phase).  Host assembles the
full output from the stripe layout.

Optimizations over the naive emission (785 -> 551 -> 396us modeled):
  - Per-chunk AllToAll pieces (15us fixed + size/40GBps each in the cost
    model) overlap the collective device with the main loop; the mixing
    (gate + mix matmuls for the core's 64-token stripes) runs in-loop at
    a 2-iteration skew for early pieces and is deferred to the tail for
    pieces 3..7, filling the PE idle while the last piece's collective
    is in flight.
  - Conv layer-2 outputs go to dedicated C3 tiles (not xg reuse), so the
    l2/phase5/bounce trail shrinks from 2 chunks to 1 with no WAR hazard
    against l0's lagged xg reads.
  - ALL conv taps run on the PE as diag matmuls; the DVE only gets each
    tap's single partial boundary-chunk segment.  (Any static or per-
    chunk-balanced reassignment of l0/l1 taps to the DVE measured WORSE:
    l1 segs at the DVE queue tail gate the next iteration's l2 base
    reads of C2, and the phase5 TTs behind big seg batches gate the
    bounce->collective path.)
  - Single-instruction batched DMAs (einops-rearranged APs) for weights,
    per-chunk x loads (prefetched two iterations ahead), the bounce, the
    mix input gather, and f16 token-major output stores.
  - The last iteration is reordered (bounce(6) -> memory(7) serial ->
    convs(7) -> bounce(7)) to race chunk 7 to its collective.
  - Conv taps are merged across heads/groups into full-width 128-row ops
    keyed by (pair, layer, lag); the head->position assignment maximizes
    lag sharing (memory heads 6,7,8,9 pinned to pair 2).
  - Layer-0/1 taps run on the tensor engine as block-diagonal [128x128]
    stationary matmuls accumulated in PSUM (base scale s1=1+w3 included),
    evicted once per chunk by the Act engine with the conv bias fused;
    layer-2 taps run on DVE per-chunk (their consumer trails by 2 chunks),
    except the last two chunks where they hop back to the then-idle PE so
    the bounce->collective path is not gated by the DVE backlog.
  - Chunk-pipelined emission with phase1 one chunk ahead (its SwiGLU TTs
    must beat the l2 seg batch into the DVE queue), l2/phase5/bounce
    trailing by two chunks, and the neural-memory recurrence split into an
    M-independent precompute (projections, gates, decay, write outer
    products staged to SBUF) plus a minimal reads-matmul/M-update chain.
  - PSUM tags are partitioned by stream (phase1/conv/memory/reads/writes)
    so buffer rotation does not serialize unrelated phases.
  - The AllToAll is split in two column-halves; the second overlaps with
    the mixing of the first, and mixing runs in four 512-token units.

Self-contained: hardcodes all shapes; builds + compiles once per process.
"""
import math

import numpy as np

DILATIONS = [(1, 2, 4), (1, 1, 1), (4, 8, 16), (8, 16, 32), (32, 64, 128),
             (64, 128, 256), (256, 512, 1024), (1, 100, 200), (1, 500, 1000),
             (1, 1024, 2048), (3, 9, 27), (5, 25, 125)]
MEM_HEADS = (6, 7, 8, 9)
HIDDEN = 768
B, S = 4, 4096
N_CORES = 8
# position-sets chosen to maximize same-lag sharing within each pair:
# p0={0,1,10,11} p1={2,3,4,5} p2={6,7,8,9} (memory heads must sit at p2)
GROUPS = [[0, 1, 2, 3, 6, 8], [10, 11, 4, 5, 7, 9]]
PERM_HEADS = GROUPS[0] + GROUPS[1]
TOK = S // N_CORES   # 512
NB = HIDDEN // 128   # 6
NCK = S // 512       # 8

_CACHE = {}


def _build_schedule():
    """Merged conv taps: one op per (pair, layer, lag) serving every
    (group, hh, k) needing that lag.  Engine-assigned to balance busy ns.

    Returns (taps, n_bias_cols, n_sc_cols, n_diag).
      tap: dict(p, l, lag, users=[(gi,hh,k)], eng in {pe,dve,pool},
                diag(int|None), col(int|None))
      diag: index into the convdiag stationary blocks (after the 9 bases)
      col:  index into conv_sc weight columns (after the 9 bias cols)
    """
    taps = []
    for p in range(3):
        for l in range(3):
            u = {}
            for gi in range(2):
                for hh in range(2):
                    h = GROUPS[gi][2 * p + hh]
                    d = DILATIONS[h][l]
                    for k in (1, 2, 3):
                        lag = k * d
                        if lag < S:
                            u.setdefault(lag, []).append((gi, hh, k))
            for lag in sorted(u):
                taps.append(dict(p=p, l=l, lag=lag, users=u[lag]))

    # Taps run on the PE as diag matmuls (full-coverage chunks); the DVE
    # gets each PE tap's single partial boundary-chunk segment (chunk
    # lag//512, cols [lag%512..)).  This keeps the DVE queue short so the
    # phase5->bounce->collective path launches mid-iteration instead of
    # being gated by a ~19us/chunk l2 seg backlog.  To rebalance engine
    # load, each chunk sends its 16 smallest-lag ACTIVE l1 taps to the
    # DVE (full-chunk segments) -- l1's consumer (l2 of the same pair)
    # trails by one iteration, so that backlog is off the bounce path
    # (unlike l0, which feeds l1 in the same iteration, and l2, which
    # feeds phase5/bounce).
    # l0 taps run fully on the DVE: their sources (xg) are ready at
    # iteration start so the segs can be emitted right after the bounce
    # in the DVE queue, and their consumer (l1's PE base) doesn't run
    # until ~27us into the iteration -- ample slack.  l1/l2 taps stay on
    # PE (l1 segs at the queue tail were shown to gate the next
    # iteration's l2 base reads; l2 segs gate phase5/bounce).
    for t in taps:
        t["eng"] = "pe"
        t["late_pe"] = False
        t["dve_chunks"] = set(range(NCK)) if t["l"] == 0 else set()

    # diag blocks ordered by layer so the l0 prefix of the (large) cdg
    # constant can be DMA'd first at startup.
    n_diag = 9
    n_cols = 9
    n_diag_l0 = None
    for t in sorted(taps, key=lambda t: (t["l"], t["p"], t["lag"])):
        t["diag"] = None
        t["col"] = None
        first = -(-t["lag"] // 512)
        if any(c not in t["dve_chunks"] for c in range(first, 8)):
            t["diag"] = n_diag
            n_diag += 1
        if t["dve_chunks"] or t["lag"] % 512:
            t["col"] = n_cols
            n_cols += 1
        if t["l"] == 0:
            n_diag_l0 = n_diag
    return taps, n_cols, n_diag, n_diag_l0


_DVE_TAP_CAP = 0
_TAPS, _N_COLS, _N_DIAG, _N_DIAG_L0 = _build_schedule()


def _build_bass(reps=1):
    import concourse.bacc as bacc
    import concourse.mybir as mybir
    import concourse.tile as tile

    f32 = mybir.dt.float32
    f16 = mybir.dt.float16
    AF = mybir.ActivationFunctionType
    OP = mybir.AluOpType

    nc = bacc.Bacc("TRN2", target_bir_lowering=False, debug=False,
                   num_devices=N_CORES)

    def din(name, shape, dt=f32):
        return nc.dram_tensor(name, shape, dt, kind="ExternalInput").ap()

    xT_d = din("xT", [HIDDEN, S], f16)
    wgT_d = din("wgT", [HIDDEN, HIDDEN], f16)
    rT_d = din("rT", [HIDDEN, 8], f16)
    rb_d = din("rb", [8, 1])
    csc_d = din("conv_sc", [128, _N_COLS])
    cdg_d = din("conv_diag", [128, 128 * _N_DIAG], f16)
    qbd_d = din("mem_qbd", [128, 128], f16)
    kvg_d = din("mem_kvg", [128, 386], f16)
    gbb_d = din("mem_gb_bc", [128, 2])
    wot_d = din("mem_WoT", [128, 256], f16)
    ones_d = din("ones64", [128, 64])
    eye_d = din("eye64", [64, 64], f16)
    eind_d = din("E_ind", [8, 384], f16)
    mgT_d = din("mixgT", [HIDDEN, HIDDEN], f16)
    mgb_d = din("mixgb", [HIDDEN, 1])
    mxT_d = din("mixT", [HIDDEN, HIDDEN], f16)
    mxb_d = din("mixb_bc", [128, HIDDEN])
    y_d = nc.dram_tensor("y", [B * TOK, HIDDEN], f16, kind="ExternalOutput").ap()

    with tile.TileContext(nc) as tc:
        with (
            tc.tile_pool(name="const", bufs=1) as constp,
            tc.tile_pool(name="main", bufs=1) as mainp,
            tc.tile_pool(name="xt", bufs=2) as xtp,
            tc.tile_pool(name="tmp", bufs=3) as tmpp,
            tc.tile_pool(name="ps", bufs=2, space="PSUM") as psp,
            tc.tile_pool(name="dram", bufs=1, space="DRAM") as dramp,
        ):
            # ---------------- resident weights / constants ----------------
            # weight blocks live in single wide tiles loaded by ONE DMA each
            # (each DMA instruction costs ~625ns of serialized HWDGE time,
            # so count matters at startup).
            wg1 = constp.tile([128, NB * HIDDEN], f16, name="wg1")
            rt1 = constp.tile([128, NB * 8], f16, name="rt1")
            wg_sb = [wg1[:, HIDDEN * i:HIDDEN * (i + 1)] for i in range(NB)]
            rT_sb = [rt1[:, 8 * i:8 * (i + 1)] for i in range(NB)]
            rb_sb = constp.tile([8, 1], f32, name="rb")

            def load_p1_weights():
                # router weights first (phase1's first matmul), then gate
                nc.sync.dma_start(rt1[:].rearrange("r (i c) -> r i c", i=NB),
                                  rT_d[:].rearrange("(i r) c -> r i c", i=NB))
                nc.sync.dma_start(rb_sb[:], rb_d[:])
                nc.sync.dma_start(wg1[:].rearrange("r (i c) -> r i c", i=NB),
                                  wgT_d[:].rearrange("(i r) c -> r i c", i=NB))
            # conv/memory constants are not needed until after phase1(0):
            # defer their DMAs behind the first xt loads so the tensor
            # engine is not stalled ~18us at startup behind the 1.7MB cdg.
            csc_sb = constp.tile([128, _N_COLS], f32, name="csc")
            cdg_sb = constp.tile([128, 128 * _N_DIAG], f16, name="cdg")
            qbd_sb = constp.tile([128, 128], f16, name="qbd")
            kvg_sb = constp.tile([128, 386], f16, name="kvgw")
            gbb_sb = constp.tile([128, 2], f32, name="gbb")
            wot_sb = constp.tile([128, 256], f16, name="wot")
            ones_sb = constp.tile([128, 64], f32, name="ones")
            eye64_sb = constp.tile([64, 64], f16, name="eye64")
            eind_sb = constp.tile([8, 384], f16, name="eind")

            def load_deferred_consts_a():
                # needed during iteration 0: memory projections, conv l0
                nc.sync.dma_start(qbd_sb[:], qbd_d[:])
                nc.sync.dma_start(kvg_sb[:], kvg_d[:])
                nc.sync.dma_start(gbb_sb[:], gbb_d[:])
                nc.sync.dma_start(csc_sb[:], csc_d[:])
                ca = 128 * _N_DIAG_L0
                nc.sync.dma_start(cdg_sb[:, :ca], cdg_d[:, :ca])
                nc.sync.dma_start(ones_sb[:], ones_d[:])
                nc.sync.dma_start(eye64_sb[:], eye_d[:])

            def load_deferred_consts_b():
                # needed from mid-iteration 0 (l1) / iteration 1 (phase5)
                ca = 128 * _N_DIAG_L0
                nc.sync.dma_start(cdg_sb[:, ca:], cdg_d[:, ca:])
                nc.sync.dma_start(eind_sb[:], eind_d[:])
                nc.sync.dma_start(wot_sb[:], wot_d[:])
            # mixing weights are only needed post-collective: tiles are
            # allocated here but their DMAs are deferred to after the main
            # loop so startup DMA bandwidth goes to compute-critical loads.
            mg1 = constp.tile([128, NB * HIDDEN], f16, name="mg1")
            mx1 = constp.tile([128, NB * HIDDEN], f16, name="mx1")
            mgT_sb = [mg1[:, HIDDEN * i:HIDDEN * (i + 1)] for i in range(NB)]
            mxT_sb = [mx1[:, HIDDEN * i:HIDDEN * (i + 1)] for i in range(NB)]
            mgb_sb = constp.tile([128, NB], f32, name="mgb")
            mxb_sb = constp.tile([128, HIDDEN], f32, name="mxb")

            def load_mix_weights():
                nc.sync.dma_start(mg1[:].rearrange("r (i c) -> r i c", i=NB),
                                  mgT_d[:].rearrange("(i r) c -> r i c", i=NB))
                nc.sync.dma_start(mx1[:].rearrange("r (i c) -> r i c", i=NB),
                                  mxT_d[:].rearrange("(i r) c -> r i c", i=NB))
                nc.sync.dma_start(mgb_sb[:].rearrange("r (i c) -> r i c", i=NB),
                                  mgb_d[:].rearrange("(i r) c -> r i c", i=NB))
                nc.sync.dma_start(mxb_sb[:], mxb_d[:])

            def diag(i):
                return cdg_sb[:, 128 * i:128 * (i + 1)]

            # ---------------- persistent state (per rep) ----------------
            for _rep in range(reps):
              xg = [mainp.tile([128, S], f16, name=f"xg{p}", tag=f"xg{p}") for p in range(3)]
              C1 = [mainp.tile([128, S], f16, name=f"c1_{p}", tag=f"c1_{p}") for p in range(3)]
              C2 = [mainp.tile([128, S], f16, name=f"c2_{p}", tag=f"c2_{p}") for p in range(3)]
              # per-chunk router weights / memory output, 3-deep rings
              # (consumers trail producers by exactly 2 chunks)
              hw_t = {}
              mem_t = {}
              _mem_stash = {}
              rd_ck = [mainp.tile([128, 512], f16, name=f"rdck{h}", tag=f"rdck{h}") for h in range(2)]
              mprev_t = {}
              mprev_t[0] = tmpp.tile([64, 256], f16, name="mprev", tag="mprev",
                                     bufs=2)
              nc.vector.memset(mprev_t[0][:], 0.0)

              # conv chains: layer l: src CH[p][l] -> dst CH[p][l+1].
              # All three pairs get dedicated l2 destination tiles so the
              # l2/phase5/bounce trail can shrink to 1 chunk without WAR
              # hazards against l0's lagged xg reads (and xg2 stays intact
              # for the memory phase).
              C3 = [mainp.tile([128, S], f16, name=f"c3_{p}", tag=f"c3_{p}")
                    for p in range(3)]
              CH = [[xg[p], C1[p], C2[p], C3[p]] for p in range(3)]
              FINAL = [CH[p][3] for p in range(3)]

              def emit_sc_tap(t, c):
                  """DVE/Pool tap segment for dst chunk c: cols [max(lag,
                  512c), 512(c+1))."""
                  lo, hi = max(t["lag"], 512 * c), 512 * (c + 1)
                  if lo >= hi:
                      return
                  src, dst = CH[t["p"]][t["l"]], CH[t["p"]][t["l"] + 1]
                  e = nc.gpsimd if t["eng"] == "pool" else nc.vector
                  c_ = t["col"]
                  e.scalar_tensor_tensor(
                      dst[:, lo:hi], src[:, lo - t["lag"]:hi - t["lag"]],
                      csc_sb[:, c_:c_ + 1], dst[:, lo:hi], OP.mult, OP.add)

              def emit_conv(p, l, c, part="all"):
                  """One (pair, layer) chunk: PE-accumulated taps + eviction
                  with bias, then per-chunk DVE tap segments.  part="base"
                  emits only the PE matmuls + eviction; part="segs" only the
                  DVE segments (so l0's eviction can be queued early while
                  its segs sit later in the DVE queue)."""
                  cs_ = slice(512 * c, 512 * (c + 1))
                  src, dst = CH[p][l], CH[p][l + 1]

                  def on_pe(t):
                      return 512 * c >= t["lag"] and c not in t["dve_chunks"]

                  if part in ("all", "base"):
                      ps_c = psp.tile([128, 512], f32, name="psc", tag="B")
                      pe_taps = [t for t in _TAPS
                                 if t["p"] == p and t["l"] == l and on_pe(t)]
                      nc.tensor.matmul(ps_c[:], diag(3 * p + l), src[:, cs_],
                                       start=True, stop=not pe_taps)
                      for i, t in enumerate(pe_taps):
                          a = 512 * c - t["lag"]
                          nc.tensor.matmul(ps_c[:], diag(t["diag"]),
                                           src[:, a:a + 512], start=False,
                                           stop=(i == len(pe_taps) - 1))
                      nc.scalar.activation(
                          dst[:, cs_], ps_c[:], AF.Identity,
                          bias=csc_sb[:, 3 * p + l:3 * p + l + 1], scale=1.0)
                  if part in ("all", "segs"):
                      for t in _TAPS:
                          if t["p"] != p or t["l"] != l or on_pe(t):
                              continue
                          if (c in t["dve_chunks"]
                                  or (t["lag"] % 512 and t["lag"] // 512 == c)):
                              emit_sc_tap(t, c)

              def emit_phase5(c):
                  cs_ = slice(512 * c, 512 * (c + 1))
                  nc.vector.tensor_tensor(FINAL[2][:, cs_], FINAL[2][:, cs_],
                                          mem_t[c][:], OP.add)
                  for p in range(3):
                      ps_h = psp.tile([128, 512], f32, name="psh", tag="B")
                      nc.tensor.matmul(ps_h[:], eind_sb[:, 128 * p:128 * (p + 1)],
                                       hw_t[c][:], start=True, stop=True)
                      nc.vector.tensor_tensor(FINAL[p][:, cs_], FINAL[p][:, cs_],
                                              ps_h[:], OP.mult)

              # per-chunk exchange buffers: [8 dst-core blocks x 384, 64].
              # Core i's block j = its pair features for the 64-token
              # stripe [512c+64j, 512c+64j+64); the AllToAll hands core j
              # that stripe from every core.
              bnc = [dramp.tile([N_CORES * 384, 64], f16, name=f"bin{c}")
                     for c in range(NCK)]
              bnco = [dramp.tile([N_CORES * 384, 64], f16, name=f"bout{c}")
                      for c in range(NCK)]

              def emit_bounce(c):
                  # dst view [384, 8, 64]: (x, j, t) -> dram row 384j+x
                  dv = bnc[c][:].rearrange("(j x) t -> x j t", j=N_CORES)
                  for p in range(3):
                      nc.sync.dma_start(
                          dv[128 * p:128 * (p + 1), :, :],
                          FINAL[p][:, 512 * c:512 * (c + 1)]
                          .rearrange("r (j t) -> r j t", j=N_CORES))
                  nc.gpsimd.collective_compute(
                      "AllToAll", mybir.AluOpType.bypass,
                      replica_groups=[list(range(N_CORES))],
                      ins=[bnc[c][:].opt()], outs=[bnco[c][:].opt()])

              def emit_mix(c):
                  """Mix the core's 64-token stripe of chunk c (4 batches x
                  64 tokens = 256 columns) from the landed piece bnco[c]."""
                  htb = tmpp.tile([128, NB * 256], f16, name="htb", tag="htb",
                                  bufs=1)
                  ht = [htb[:, 256 * i:256 * (i + 1)] for i in range(NB)]
                  gh = [tmpp.tile([128, 256], f16, name=f"ghu{i}",
                                  tag=f"ghu{i}", bufs=1) for i in range(NB)]
                  # per-batch DMA (3-dim AP limit): src rows (2b+g)*384 +
                  # 128p + r -> dst block fb=3g+p, col 64b+t
                  dvw = htb[:].rearrange("r (fb b t) -> r fb b t", fb=NB, b=4)
                  svw = bnco[c][:].rearrange("(b fb r) t -> r b fb t",
                                             b=4, fb=NB)
                  for b in range(4):
                      nc.sync.dma_start(dvw[:, :, b, :], svw[:, b, :, :])
                  for fb in range(NB):
                      ps_pre = psp.tile([128, 256], f32, name="pre", tag="A")
                      for db in range(NB):
                          nc.tensor.matmul(ps_pre[:],
                                           mgT_sb[db][:, 128 * fb:128 * (fb + 1)],
                                           ht[db][:], start=(db == 0),
                                           stop=(db == NB - 1))
                      sg = tmpp.tile([128, 256], f16, name="msig", tag="msig",
                                     bufs=2)
                      nc.scalar.activation(sg[:], ps_pre[:], AF.Sigmoid,
                                           bias=mgb_sb[:, fb:fb + 1], scale=1.0)
                      nc.vector.tensor_tensor(gh[fb][:], ht[fb][:], sg[:],
                                              OP.mult)
                  y_sb = tmpp.tile([128, 2 * HIDDEN], f16, name="ysb",
                                   tag="ysb", bufs=1)
                  for tb in range(2):
                      tr = slice(128 * tb, 128 * (tb + 1))
                      for half in range(2):
                          ps_y = psp.tile([128, 384], f32, name="psy",
                                          tag=("B" if half == 0 else "C"))
                          for fb in range(NB):
                              nc.tensor.matmul(
                                  ps_y[:], gh[fb][:, tr],
                                  mxT_sb[fb][:, 384 * half:384 * (half + 1)],
                                  start=(fb == 0), stop=(fb == NB - 1))
                          yc = HIDDEN * tb + 384 * half
                          nc.vector.tensor_tensor(
                              y_sb[:, yc:yc + 384], ps_y[:],
                              mxb_sb[:, 384 * half:384 * (half + 1)], OP.add)
                      nc.sync.dma_start(
                          y_d[256 * c + 128 * tb:256 * c + 128 * (tb + 1), :],
                          y_sb[:, HIDDEN * tb:HIDDEN * (tb + 1)])

              # Persistent PSUM memory state: ps_M holds the decay-rescaled
              # state M~ = M_0 + sum_j W_j/c_{j+1} (c_h = prod_{j<h} d_j, all
              # per-chunk), accumulated purely by PE matmuls -- the decay
              # multiply is folded into the staged kg (x 1/c_{h+1}) and the
              # per-half Act copy-out applies the c_h rescale.  This removes
              # the per-half DVE M-update entirely, so the serial chain is a
              # PE<->Act ping-pong and never blocks the DVE queue.  Gates
              # are ~0.1-0.16 here so d in [0.84, 0.9] and 1/c_8 <= 4: safe.
              ps_M = psp.tile([64, 256], f32, name="psM", tag="E", bufs=1)

              def emit_memory(ck):
                  x_mem = xg[2]
                  cs_ = slice(512 * ck, 512 * (ck + 1))
                  # --- (1) M-independent precompute ---
                  ps_qa = psp.tile([64, 512], f32, name="psqa", tag="C")
                  nc.tensor.matmul(ps_qa[:], qbd_sb[:, 0:64], x_mem[:, cs_],
                                   start=True, stop=True)
                  q_a = tmpp.tile([64, 512], f16, name="qa", tag="qa", bufs=2)
                  nc.scalar.copy(q_a[:], ps_qa[:])
                  ps_qb = psp.tile([64, 512], f32, name="psqb", tag="C")
                  nc.tensor.matmul(ps_qb[:], qbd_sb[:, 64:128], x_mem[:, cs_],
                                   start=True, stop=True)
                  q_b = tmpp.tile([64, 512], f16, name="qb", tag="qb", bufs=2)
                  nc.scalar.copy(q_b[:], ps_qb[:])
                  kg8 = tmpp.tile([64, 1024], f16, name="kg8", tag="kg8", bufs=2)
                  v8 = tmpp.tile([64, 2048], f16, name="v8", tag="v8", bufs=2)
                  dec8 = tmpp.tile([64, 16], f32, name="dec8", tag="dec8", bufs=2)
                  # decay prefix products, built incrementally (ci[h] only
                  # needs decays through half h, so everything stays in one
                  # loop and ps_kvg is consumed before its buffer recycles):
                  # ch8[2h+hd] = c_h (h=0..8), ci8[2h+hd] = 1/c_{h+1}
                  inv8 = tmpp.tile([64, 16], f32, name="inv8", tag="inv8", bufs=2)
                  ch8 = tmpp.tile([64, 18], f32, name="ch8", tag="ch8", bufs=2)
                  ci8 = tmpp.tile([64, 16], f32, name="ci8", tag="ci8", bufs=2)
                  # [64, 512]: head-a kT blocks in cols 0:256, head-b in
                  # 256:512 (both at base partition 0 for the S matmuls)
                  nc.vector.memset(ch8[:, 0:2], 1.0)
                  for h in range(8):
                      c64 = slice(512 * ck + 64 * h, 512 * ck + 64 * (h + 1))
                      ps_kvg = psp.tile([64, 386], f32, name="pskvg", tag="C")
                      nc.tensor.matmul(ps_kvg[:], x_mem[:, c64], kvg_sb[:],
                                       start=True, stop=True)
                      g_sb = tmpp.tile([64, 2], f32, name="gsb", tag="gsb")
                      for hh in range(2):
                          nc.scalar.activation(g_sb[:, hh:hh + 1],
                                               ps_kvg[:, 384 + hh:385 + hh],
                                               AF.Sigmoid,
                                               bias=gbb_sb[0:64, hh:hh + 1],
                                               scale=1.0)
                      nc.scalar.copy(v8[:, 256 * h:256 * (h + 1)],
                                     ps_kvg[:, 128:384])
                      ps_g = psp.tile([64, 2], f32, name="psg", tag="C")
                      nc.tensor.matmul(ps_g[:], ones_sb[0:64, :], g_sb[:],
                                       start=True, stop=True)
                      nc.scalar.activation(dec8[:, 2 * h:2 * h + 2], ps_g[:],
                                           AF.Identity, bias=1.0, scale=-1.0)
                      nc.vector.reciprocal(inv8[:, 2 * h:2 * h + 2],
                                           dec8[:, 2 * h:2 * h + 2])
                      if h == 0:
                          nc.vector.tensor_copy(ci8[:, 0:2], inv8[:, 0:2])
                      else:
                          nc.vector.tensor_tensor(ch8[:, 2 * h:2 * h + 2],
                                                  ch8[:, 2 * h - 2:2 * h],
                                                  dec8[:, 2 * h - 2:2 * h],
                                                  OP.mult)
                          nc.vector.tensor_tensor(ci8[:, 2 * h:2 * h + 2],
                                                  ci8[:, 2 * h - 2:2 * h],
                                                  inv8[:, 2 * h:2 * h + 2],
                                                  OP.mult)
                      gt = tmpp.tile([64, 2], f32, name="gt", tag="gt")
                      nc.vector.tensor_tensor(gt[:], g_sb[:],
                                              ci8[:, 2 * h:2 * h + 2], OP.mult)
                      for hh in range(2):
                          nc.vector.tensor_scalar(
                              kg8[:, 128 * h + 64 * hh:128 * h + 64 * (hh + 1)],
                              ps_kvg[:, 64 * hh:64 * (hh + 1)],
                              gt[:, hh:hh + 1], None, OP.mult)
                  nc.vector.tensor_tensor(ch8[:, 16:18], ch8[:, 14:16],
                                          dec8[:, 14:16], OP.mult)
                  _mem_stash[ck] = (q_a, q_b, kg8, v8, ch8)

              def emit_memory_serial(ck):
                  q_a, q_b, kg8, v8, ch8 = _mem_stash[ck]
                  # --- (2) serial recurrence: seed ps_M from the previous
                  # chunk's state, then per half: Act copy-out (applying the
                  # c_h rescale), reads-matmul, and W accumulation.
                  mprev = mprev_t[ck]
                  for hd in range(2):
                      nc.tensor.matmul(ps_M[:, 128 * hd:128 * (hd + 1)],
                                       eye64_sb[:], mprev[:, 128 * hd:128 * (hd + 1)],
                                       start=True, stop=True)
                  mnext = tmpp.tile([64, 256], f16, name="mprev", tag="mprev",
                                    bufs=2)
                  mprev_t[ck + 1] = mnext
                  for h in range(8):
                      half = h % 2
                      if half == 0:
                          ps_rd = psp.tile([128, 256], f32, name="psrd", tag="D", bufs=1)
                      if h == 0:
                          m_sb = mprev
                      else:
                          m_sb = tmpp.tile([64, 256], f16, name="msb", tag="msb",
                                           bufs=2)
                          for hd in range(2):
                              nc.scalar.activation(
                                  m_sb[:, 128 * hd:128 * (hd + 1)],
                                  ps_M[:, 128 * hd:128 * (hd + 1)], AF.Identity,
                                  scale=ch8[:, 2 * h + hd:2 * h + hd + 1])
                      nc.tensor.matmul(ps_rd[:, 64 * half:64 * (half + 1)],
                                       m_sb[:, 0:128], q_a[:, 64 * h:64 * (h + 1)],
                                       start=True, stop=True)
                      nc.tensor.matmul(ps_rd[:, 128 + 64 * half:128 + 64 * (half + 1)],
                                       m_sb[:, 128:256], q_b[:, 64 * h:64 * (h + 1)],
                                       start=True, stop=True)
                      nc.tensor.matmul(ps_M[:, 0:128],
                                       kg8[:, 128 * h:128 * h + 64],
                                       v8[:, 256 * h:256 * h + 128],
                                       start=False, stop=True)
                      nc.tensor.matmul(ps_M[:, 128:256],
                                       kg8[:, 128 * h + 64:128 * (h + 1)],
                                       v8[:, 256 * h + 128:256 * (h + 1)],
                                       start=False, stop=True)
                      if half == 1:
                          blk = 4 * ck + h // 2
                          cc = 128 * blk % 512
                          for hh in range(2):
                              nc.scalar.copy(rd_ck[hh][:, cc:cc + 128],
                                             ps_rd[:, 128 * hh:128 * (hh + 1)])
                  for hd in range(2):
                      nc.scalar.activation(
                          mnext[:, 128 * hd:128 * (hd + 1)],
                          ps_M[:, 128 * hd:128 * (hd + 1)], AF.Identity,
                          scale=ch8[:, 16 + hd:17 + hd])
                  ps_o = psp.tile([128, 512], f32, name="pso", tag="C")
                  nc.tensor.matmul(ps_o[:], wot_sb[:, 0:128], rd_ck[0][:],
                                   start=True, stop=False)
                  nc.tensor.matmul(ps_o[:], wot_sb[:, 128:256], rd_ck[1][:],
                                   start=False, stop=True)
                  mem_t[ck] = tmpp.tile([128, 512], f16, name="memo",
                                        tag="memo")
                  nc.scalar.copy(mem_t[ck][:], ps_o[:])

              # ======== main chunk-pipelined driver ========
              # Phase1 runs one chunk AHEAD of everything else so its SwiGLU
              # TTs enter the DVE queue before the previous chunk's l2 seg
              # batch (otherwise the next iteration's PE work -- memory
              # projections, conv l0 -- stalls ~7us per chunk waiting for
              # xg).  memory/l0/l1 at chunk ck; l2 + phase5 + bounce trail
              # by 2 chunks (xg reuse as l2 dst needs l0 lags <= 1024).
              xtt_t = {}

              def load_xt(ck):
                  cs = slice(512 * ck, 512 * (ck + 1))
                  xtt = xtp.tile([128, NB * 512], f16, name="xtt", tag="xtt")
                  nc.sync.dma_start(
                      xtt[:].rearrange("r (i c) -> r i c", i=NB),
                      xT_d[:, cs].rearrange("(i r) c -> r i c", i=NB))
                  xtt_t[ck] = xtt

              def emit_phase1(ck):
                  cs = slice(512 * ck, 512 * (ck + 1))
                  xtt = xtt_t.pop(ck)
                  xt = [xtt[:, 512 * i:512 * (i + 1)] for i in range(NB)]
                  ps_r = psp.tile([8, 512], f32, name="psr", tag="C")
                  for db in range(NB):
                      nc.tensor.matmul(ps_r[:], rT_sb[db][:], xt[db][:],
                                       start=(db == 0), stop=(db == NB - 1))
                  hw_t[ck] = tmpp.tile([8, 512], f16, name="hww", tag="hww",
                                       bufs=4)
                  nc.scalar.activation(hw_t[ck][:], ps_r[:], AF.Sigmoid,
                                       bias=rb_sb[:, 0:1], scale=1.0)
                  for pb in range(3):
                      ps_a = psp.tile([128, 512], f32, name="psa", tag="A")
                      ps_b = psp.tile([128, 512], f32, name="psb", tag="B")
                      for db in range(NB):
                          nc.tensor.matmul(
                              ps_a[:], wg_sb[db][:, 128 * pb:128 * (pb + 1)],
                              xt[db][:], start=(db == 0), stop=(db == NB - 1))
                      for db in range(NB):
                          nc.tensor.matmul(
                              ps_b[:],
                              wg_sb[db][:, 384 + 128 * pb:384 + 128 * (pb + 1)],
                              xt[db][:], start=(db == 0), stop=(db == NB - 1))
                      sig = tmpp.tile([128, 512], f16, name="sig", tag="sig", bufs=2)
                      nc.scalar.activation(sig[:], ps_b[:], AF.Sigmoid)
                      nc.vector.tensor_tensor(xg[pb][:, cs], ps_a[:], sig[:],
                                              OP.mult)

              load_xt(0)
              if _rep == 0:
                  load_p1_weights()
              load_xt(1)
              emit_phase1(0)
              if _rep == 0:
                  load_deferred_consts_a()
              for ck in range(NCK):
                  if ck + 2 < NCK:
                      load_xt(ck + 2)
                  if ck + 1 < NCK:
                      emit_phase1(ck + 1)
                  if _rep == 0 and ck == 0:
                      load_deferred_consts_b()
                  if _rep == 0 and ck == 1:
                      load_mix_weights()
                  if ck < NCK - 1:
                      # ---- memory precompute (M-independent) ----
                      emit_memory(ck)
                      # ---- trailing: l2/phase5/bounce+collective at ck-1
                      # (before l0/l1 so phase5's DVE TTs are not stuck
                      # behind conv segs and the piece's collective starts
                      # early), mixing of the piece landed two iterations
                      # ago ----
                      if ck >= 1:
                          for p in range(3):
                              emit_conv(p, 2, ck - 1)
                          emit_phase5(ck - 1)
                          emit_bounce(ck - 1)
                      # mixes 3+ are deferred to the tail, where the PE
                      # otherwise idles waiting for the last piece's
                      # collective
                      if 2 <= ck <= 4:
                          emit_mix(ck - 2)
                      for p in range(3):
                          emit_conv(p, 0, ck)
                      for p in range(3):
                          emit_conv(p, 1, ck)
                      # ---- memory serial recurrence LAST: its cross-
                      # engine ping-pong then blocks only the tails of the
                      # PE/DVE queues, not ready l2-seg/phase5 bulk work --
                      emit_memory_serial(ck)
                  else:
                      # last iteration: race the chunk-7 chain (memory
                      # serial, convs, bounce) to the collective as early
                      # as possible; the deferred mixes then fill the PE
                      # while piece 7 is in flight.
                      for p in range(3):
                          emit_conv(p, 2, ck - 1)
                      emit_phase5(ck - 1)
                      emit_bounce(ck - 1)
                      emit_memory(ck)
                      emit_memory_serial(ck)
                      for p in range(3):
                          emit_conv(p, 0, ck)
                      for p in range(3):
                          emit_conv(p, 1, ck)
                      for p in range(3):
                          emit_conv(p, 2, ck)
                      emit_phase5(ck)
                      emit_bounce(ck)
                      for c in range(3, NCK):
                          emit_mix(c)

    nc.compile()
    return nc


def _prep_core_inputs(core, inp):
    b, g = core // 2, core % 2
    heads = GROUPS[g]
    f32, f16 = np.float32, np.float16

    x = np.asarray(inp["x"], f32)[b]
    gate_w = np.asarray(inp["gate_w"], f32)
    rows_xg = np.concatenate([np.arange(64 * h, 64 * h + 64) for h in heads])
    W_c = np.concatenate([gate_w[rows_xg], gate_w[768 + rows_xg]], axis=0)

    rT = np.zeros((HIDDEN, 8), f32)
    rT[:, :6] = np.asarray(inp["router_w"], f32)[heads].T
    rb = np.zeros((8, 1), f32)
    rb[:6, 0] = np.asarray(inp["router_b"], f32)[heads]

    conv_w = np.asarray(inp["conv_w"], f32)
    conv_b = np.asarray(inp["conv_b"], f32)
    # conv_sc: cols 0..8 = bias per (p, l); then tap weight columns
    csc = np.zeros((128, _N_COLS), f32)
    # conv_diag: blocks 0..8 = base diag(1 + w3) per (p, l); then PE taps
    cdg = np.zeros((128, 128 * _N_DIAG), f32)
    for p in range(3):
        for l in range(3):
            for hh in range(2):
                head = heads[2 * p + hh]
                rows = slice(64 * hh, 64 * (hh + 1))
                csc[rows, 3 * p + l] = conv_b[head, l, :]
                blk = 3 * p + l
                w3 = 1.0 + conv_w[head, l, :, 3]
                idx = np.arange(64 * hh, 64 * (hh + 1))
                cdg[idx, 128 * blk + idx] = w3
    for t in _TAPS:
        for (gi, hh, k) in t["users"]:
            if gi != g:
                continue
            head = heads[2 * t["p"] + hh]
            w = conv_w[head, t["l"], :, 3 - k]
            idx = np.arange(64 * hh, 64 * (hh + 1))
            if t["diag"] is not None:
                cdg[idx, 128 * t["diag"] + idx] = w
            if t["col"] is not None:
                csc[idx, t["col"]] = w

    ma, mb = heads[4], heads[5]
    ia, ib = MEM_HEADS.index(ma), MEM_HEADS.index(mb)
    Wq = np.asarray(inp["mem_Wq"], f32)
    Wk = np.asarray(inp["mem_Wk"], f32)
    Wv = np.asarray(inp["mem_Wv"], f32)
    Wgw = np.asarray(inp["mem_Wg_w"], f32)
    Wgb = np.asarray(inp["mem_Wg_b"], f32)
    Wo = np.asarray(inp["mem_Wout"], f32)

    qbd = np.zeros((128, 128), f32)
    qbd[0:64, 0:64] = Wq[ia].T
    qbd[64:128, 64:128] = Wq[ib].T
    kvg = np.zeros((128, 386), f32)
    kvg[0:64, 0:64] = Wk[ia].T
    kvg[64:128, 64:128] = Wk[ib].T
    kvg[0:64, 128:256] = Wv[ia].T
    kvg[64:128, 256:384] = Wv[ib].T
    kvg[0:64, 384] = Wgw[ia, 0]
    kvg[64:128, 385] = Wgw[ib, 0]
    gbb = np.zeros((128, 2), f32)
    gbb[:, 0] = Wgb[ia, 0]
    gbb[:, 1] = Wgb[ib, 0]
    wot = np.zeros((128, 256), f32)
    wot[:, 0:64] = Wo[ia].T           # head-a rows 0:64 of stacked out
    wot[:, 128 + 64:256] = Wo[ib].T   # head-b rows 64:128 of stacked out

    eind = np.zeros((8, 384), f32)
    for p in range(3):
        eind[2 * p, 128 * p:128 * p + 64] = 1.0
        eind[2 * p + 1, 128 * p + 64:128 * (p + 1)] = 1.0

    pf = np.concatenate([np.arange(64 * h, 64 * h + 64) for h in PERM_HEADS])
    mixg_w = np.asarray(inp["mixg_w"], f32)
    mix_w = np.asarray(inp["mix_w"], f32)

    return {
        "xT": np.ascontiguousarray(x.T).astype(f16),
        "wgT": np.ascontiguousarray(W_c.T).astype(f16),
        "rT": rT.astype(f16), "rb": rb, "conv_sc": csc,
        "conv_diag": cdg.astype(f16),
        "mem_qbd": qbd.astype(f16), "mem_kvg": kvg.astype(f16),
        "mem_gb_bc": gbb, "mem_WoT": wot.astype(f16),
        "ones64": np.full((128, 64), 1.0 / 64.0, f32),
        "eye64": np.eye(64, dtype=f32).astype(f16),
        "E_ind": eind.astype(f16),
        "mixgT": np.ascontiguousarray(mixg_w[np.ix_(pf, pf)].T).astype(f16),
        "mixgb": np.asarray(inp["mixg_b"], f32)[pf].reshape(HIDDEN, 1).copy(),
        "mixT": np.ascontiguousarray(mix_w[:, pf].T).astype(f16),
        "mixb_bc": np.tile(np.asarray(inp["mix_b"], f32)[None, :], (128, 1)),
    }


def prep_in_maps(inputs):
    return [_prep_core_inputs(c, inputs) for c in range(N_CORES)]


def get_bass():
    if "nc" not in _CACHE:
        _CACHE["nc"] = _build_bass()
    return _CACHE["nc"]


def assemble(results):
    # core j's y rows: 256*c + 64*b + t  ->  out[b, 512*c + 64*j + t]
    out = np.zeros((B, S, HIDDEN), np.float32)
    for j in range(N_CORES):
        y = results[j]["y"].reshape(NCK, B, 64, HIDDEN)
        for c in range(NCK):
            out[:, 512 * c + 64 * j:512 * c + 64 * (j + 1), :] = y[c]
    return out


def kernel(**inputs):
    from concourse import bass_utils
    nc = get_bass()
    in_maps = prep_in_maps(inputs)
    res = bass_utils.run_bass_kernel_spmd(nc, in_maps,
                                          core_ids=list(range(N_CORES)))
    return assemble(res.results)



# revision 72
# speedup vs baseline: 1.1651x; 1.1651x over previous
"""Trainium2 Bass kernel for nn_MultiHeadDilatedState (B=4, S=4096, H=768).

Sharding: 8 cores = (batch b in 0..4) x (head-group g in 0..2); each core
runs the head phase (gate matmul + SwiGLU + dilated causal convs + neural
memory + router weighting) for its 6 heads over the full sequence in
feature-major layout.  The head->mix exchange is pipelined INTO the main
loop: core j owns the 64-token stripe [512c+64j, 512c+64j+64) of every
chunk c, so each chunk's finished slab is AllToAll'd as its own piece
(issued at iteration c+1) and mixed two iterations later while the loop
still runs.  Only the last chunk's piece + mix trail the loop (~35us tail
instead of a ~175us serial collective+mix phase).  Host assembles the
full output from the stripe layout.

Optimizations over the naive emission (785 -> 551 -> 396us modeled):
  - Per-chunk AllToAll pieces (15us fixed + size/40GBps each in the cost
    model) overlap the collective device with the main loop; the mixing
    (gate + mix matmuls for the core's 64-token stripes) runs in-loop at
    a 2-iteration skew for early pieces and is deferred to the tail for
    pieces 3..7, filling the PE idle while the last piece's collective
    is in flight.
  - Conv layer-2 outputs go to dedicated C3 tiles (not xg reuse), so the
    l2/phase5/bounce trail shrinks from 2 chunks to 1 with no WAR hazard
    against l0's lagged xg reads.
  - ALL conv taps run on the PE as diag matmuls; the DVE only gets each
    tap's single partial boundary-chunk segment.  (Any static or per-
    chunk-balanced reassignment of l0/l1 taps to the DVE measured WORSE:
    l1 segs at the DVE queue tail gate the next iteration's l2 base
    reads of C2, and the phase5 TTs behind big seg batches gate the
    bounce->collective path.)
  - Single-instruction batched DMAs (einops-rearranged APs) for weights,
    per-chunk x loads (prefetched two iterations ahead), the bounce, the
    mix input gather, and f16 token-major output stores.
  - The last iteration is reordered (bounce(6) -> memory(7) serial ->
    convs(7) -> bounce(7)) to race chunk 7 to its collective.
  - Conv taps are merged across heads/groups into full-width 128-row ops
    keyed by (pair, layer, lag); the head->position assignment maximizes
    lag sharing (memory heads 6,7,8,9 pinned to pair 2).
  - Layer-0/1 taps run on the tensor engine as block-diagonal [128x128]
    stationary matmuls accumulated in PSUM (base scale s1=1+w3 included),
    evicted once per chunk by the Act engine with the conv bias fused;
    layer-2 taps run on DVE per-chunk (their consumer trails by 2 chunks),
    except the last two chunks where they hop back to the then-idle PE so
    the bounce->collective path is not gated by the DVE backlog.
  - Chunk-pipelined emission with phase1 one chunk ahead (its SwiGLU TTs
    must beat the l2 seg batch into the DVE queue), l2/phase5/bounce
    trailing by two chunks, and the neural-memory recurrence split into an
    M-independent precompute (projections, gates, decay, write outer
    products staged to SBUF) plus a minimal reads-matmul/M-update chain.
  - PSUM tags are partitioned by stream (phase1/conv/memory/reads/writes)
    so buffer rotation does not serialize unrelated phases.
  - The AllToAll is split in two column-halves; the second overlaps with
    the mixing of the first, and mixing runs in four 512-token units.

Self-contained: hardcodes all shapes; builds + compiles once per process.
"""
import numpy as np

DILATIONS = [(1, 2, 4), (1, 1, 1), (4, 8, 16), (8, 16, 32), (32, 64, 128),
             (64, 128, 256), (256, 512, 1024), (1, 100, 200), (1, 500, 1000),
             (1, 1024, 2048), (3, 9, 27), (5, 25, 125)]
MEM_HEADS = (6, 7, 8, 9)
HIDDEN = 768
B, S = 4, 4096
N_CORES = 8
# position-sets chosen to maximize same-lag sharing within each pair:
# p0={0,1,10,11} p1={2,3,4,5} p2={6,7,8,9} (memory heads must sit at p2)
GROUPS = [[0, 1, 2, 3, 6, 8], [10, 11, 4, 5, 7, 9]]
PERM_HEADS = GROUPS[0] + GROUPS[1]
TOK = S // N_CORES   # 512
NB = HIDDEN // 128   # 6
NCK = S // 512       # 8

_CACHE = {}


def _build_schedule():
    """Merged conv taps: one op per (pair, layer, lag) serving every
    (group, hh, k) needing that lag.  Engine-assigned to balance busy ns.

    Returns (taps, n_bias_cols, n_sc_cols, n_diag).
      tap: dict(p, l, lag, users=[(gi,hh,k)], eng in {pe,dve,pool},
                diag(int|None), col(int|None))
      diag: index into the convdiag stationary blocks (after the 9 bases)
      col:  index into conv_sc weight columns (after the 9 bias cols)
    """
    taps = []
    for p in range(3):
        for l in range(3):
            u = {}
            for gi in range(2):
                for hh in range(2):
                    h = GROUPS[gi][2 * p + hh]
                    d = DILATIONS[h][l]
                    for k in (1, 2, 3):
                        lag = k * d
                        if lag < S:
                            u.setdefault(lag, []).append((gi, hh, k))
            for lag in sorted(u):
                taps.append(dict(p=p, l=l, lag=lag, users=u[lag]))

    # Taps run on the PE as diag matmuls (full-coverage chunks); the DVE
    # gets each PE tap's single partial boundary-chunk segment (chunk
    # lag//512, cols [lag%512..)).  This keeps the DVE queue short so the
    # phase5->bounce->collective path launches mid-iteration instead of
    # being gated by a ~19us/chunk l2 seg backlog.  To rebalance engine
    # load, each chunk sends its 16 smallest-lag ACTIVE l1 taps to the
    # DVE (full-chunk segments) -- l1's consumer (l2 of the same pair)
    # trails by one iteration, so that backlog is off the bounce path
    # (unlike l0, which feeds l1 in the same iteration, and l2, which
    # feeds phase5/bounce).
    # l0 taps run fully on the DVE: their sources (xg) are ready at
    # iteration start so the segs can be emitted right after the bounce
    # in the DVE queue, and their consumer (l1's PE base) doesn't run
    # until ~27us into the iteration -- ample slack.  l1/l2 taps stay on
    # PE (l1 segs at the queue tail were shown to gate the next
    # iteration's l2 base reads; l2 segs gate phase5/bounce).
    for t in taps:
        t["eng"] = "pe"
        t["late_pe"] = False
        t["dve_chunks"] = set(range(NCK)) if t["l"] == 0 else set()

    # diag blocks ordered by layer so the l0 prefix of the (large) cdg
    # constant can be DMA'd first at startup.
    n_diag = 9
    n_cols = 9
    n_diag_l0 = None
    for t in sorted(taps, key=lambda t: (t["l"], t["p"], t["lag"])):
        t["diag"] = None
        t["col"] = None
        first = -(-t["lag"] // 512)
        if any(c not in t["dve_chunks"] for c in range(first, 8)):
            t["diag"] = n_diag
            n_diag += 1
        if t["dve_chunks"] or t["lag"] % 512:
            t["col"] = n_cols
            n_cols += 1
        if t["l"] == 0:
            n_diag_l0 = n_diag
    return taps, n_cols, n_diag, n_diag_l0


_DVE_TAP_CAP = 0
_TAPS, _N_COLS, _N_DIAG, _N_DIAG_L0 = _build_schedule()


def _build_bass(reps=1):
    import concourse.bacc as bacc
    import concourse.mybir as mybir
    import concourse.tile as tile

    f32 = mybir.dt.float32
    f16 = mybir.dt.float16
    AF = mybir.ActivationFunctionType
    OP = mybir.AluOpType

    nc = bacc.Bacc("TRN2", target_bir_lowering=False, debug=False,
                   num_devices=N_CORES)

    def din(name, shape, dt=f32):
        return nc.dram_tensor(name, shape, dt, kind="ExternalInput").ap()

    xT_d = din("xT", [HIDDEN, S], f16)
    wgT_d = din("wgT", [HIDDEN, HIDDEN], f16)
    rT_d = din("rT", [HIDDEN, 8], f16)
    rb_d = din("rb", [8, 1])
    csc_d = din("conv_sc", [128, _N_COLS])
    cdg_d = din("conv_diag", [128, 128 * _N_DIAG], f16)
    qbd_d = din("mem_qbd", [128, 128], f16)
    kvg_d = din("mem_kvg", [128, 386], f16)
    gbb_d = din("mem_gb_bc", [128, 2])
    wot_d = din("mem_WoT", [128, 256], f16)
    ones_d = din("ones64", [128, 64])
    eye_d = din("eye64", [64, 64], f16)
    eind_d = din("E_ind", [8, 384], f16)
    mgT_d = din("mixgT", [HIDDEN, HIDDEN], f16)
    mgb_d = din("mixgb", [HIDDEN, 1])
    mxT_d = din("mixT", [HIDDEN, HIDDEN], f16)
    mxb_d = din("mixb_bc", [128, HIDDEN])
    y_d = nc.dram_tensor("y", [B * TOK, HIDDEN], f16, kind="ExternalOutput").ap()

    with tile.TileContext(nc) as tc:
        with (
            tc.tile_pool(name="const", bufs=1) as constp,
            tc.tile_pool(name="main", bufs=1) as mainp,
            tc.tile_pool(name="xt", bufs=2) as xtp,
            tc.tile_pool(name="tmp", bufs=3) as tmpp,
            tc.tile_pool(name="ps", bufs=2, space="PSUM") as psp,
            tc.tile_pool(name="dram", bufs=1, space="DRAM") as dramp,
        ):
            # ---------------- resident weights / constants ----------------
            # weight blocks live in single wide tiles loaded by ONE DMA each
            # (each DMA instruction costs ~625ns of serialized HWDGE time,
            # so count matters at startup).
            wg1 = constp.tile([128, NB * HIDDEN], f16, name="wg1")
            rt1 = constp.tile([128, NB * 8], f16, name="rt1")
            wg_sb = [wg1[:, HIDDEN * i:HIDDEN * (i + 1)] for i in range(NB)]
            rT_sb = [rt1[:, 8 * i:8 * (i + 1)] for i in range(NB)]
            rb_sb = constp.tile([8, 1], f32, name="rb")

            def load_p1_weights():
                # router weights first (phase1's first matmul), then gate
                nc.sync.dma_start(rt1[:].rearrange("r (i c) -> r i c", i=NB),
                                  rT_d[:].rearrange("(i r) c -> r i c", i=NB))
                nc.sync.dma_start(rb_sb[:], rb_d[:])
                nc.sync.dma_start(wg1[:].rearrange("r (i c) -> r i c", i=NB),
                                  wgT_d[:].rearrange("(i r) c -> r i c", i=NB))
            # conv/memory constants are not needed until after phase1(0):
            # defer their DMAs behind the first xt loads so the tensor
            # engine is not stalled ~18us at startup behind the 1.7MB cdg.
            csc_sb = constp.tile([128, _N_COLS], f32, name="csc")
            cdg_sb = constp.tile([128, 128 * _N_DIAG], f16, name="cdg")
            qbd_sb = constp.tile([128, 128], f16, name="qbd")
            kvg_sb = constp.tile([128, 386], f16, name="kvgw")
            gbb_sb = constp.tile([128, 2], f32, name="gbb")
            wot_sb = constp.tile([128, 256], f16, name="wot")
            ones_sb = constp.tile([128, 64], f32, name="ones")
            eye64_sb = constp.tile([64, 64], f16, name="eye64")
            eind_sb = constp.tile([8, 384], f16, name="eind")

            def load_deferred_consts_a():
                # needed during iteration 0: memory projections, conv l0
                nc.sync.dma_start(qbd_sb[:], qbd_d[:])
                nc.sync.dma_start(kvg_sb[:], kvg_d[:])
                nc.sync.dma_start(gbb_sb[:], gbb_d[:])
                nc.sync.dma_start(csc_sb[:], csc_d[:])
                ca = 128 * _N_DIAG_L0
                nc.sync.dma_start(cdg_sb[:, :ca], cdg_d[:, :ca])
                nc.sync.dma_start(ones_sb[:], ones_d[:])
                nc.sync.dma_start(eye64_sb[:], eye_d[:])

            def load_deferred_consts_b():
                # needed from mid-iteration 0 (l1) / iteration 1 (phase5)
                ca = 128 * _N_DIAG_L0
                nc.sync.dma_start(cdg_sb[:, ca:], cdg_d[:, ca:])
                nc.sync.dma_start(eind_sb[:], eind_d[:])
                nc.sync.dma_start(wot_sb[:], wot_d[:])
            # mixing weights are only needed post-collective: tiles are
            # allocated here but their DMAs are deferred to after the main
            # loop so startup DMA bandwidth goes to compute-critical loads.
            mg1 = constp.tile([128, NB * HIDDEN], f16, name="mg1")
            mx1 = constp.tile([128, NB * HIDDEN], f16, name="mx1")
            mgT_sb = [mg1[:, HIDDEN * i:HIDDEN * (i + 1)] for i in range(NB)]
            mxT_sb = [mx1[:, HIDDEN * i:HIDDEN * (i + 1)] for i in range(NB)]
            mgb_sb = constp.tile([128, NB], f32, name="mgb")
            mxb_sb = constp.tile([128, HIDDEN], f32, name="mxb")

            def load_mix_weights():
                nc.sync.dma_start(mg1[:].rearrange("r (i c) -> r i c", i=NB),
                                  mgT_d[:].rearrange("(i r) c -> r i c", i=NB))
                nc.sync.dma_start(mx1[:].rearrange("r (i c) -> r i c", i=NB),
                                  mxT_d[:].rearrange("(i r) c -> r i c", i=NB))
                nc.sync.dma_start(mgb_sb[:].rearrange("r (i c) -> r i c", i=NB),
                                  mgb_d[:].rearrange("(i r) c -> r i c", i=NB))
                nc.sync.dma_start(mxb_sb[:], mxb_d[:])

            def diag(i):
                return cdg_sb[:, 128 * i:128 * (i + 1)]

            # ---------------- persistent state (per rep) ----------------
            for _rep in range(reps):
              xg = [mainp.tile([128, S], f16, name=f"xg{p}", tag=f"xg{p}") for p in range(3)]
              C1 = [mainp.tile([128, S], f16, name=f"c1_{p}", tag=f"c1_{p}") for p in range(3)]
              C2 = [mainp.tile([128, S], f16, name=f"c2_{p}", tag=f"c2_{p}") for p in range(3)]
              # per-chunk router weights / memory output, 3-deep rings
              # (consumers trail producers by exactly 2 chunks)
              hw_t = {}
              mem_t = {}
              _mem_stash = {}
              rd_ck = [mainp.tile([128, 512], f16, name=f"rdck{h}", tag=f"rdck{h}") for h in range(2)]
              mprev_t = {}
              mprev_t[0] = tmpp.tile([64, 256], f16, name="mprev", tag="mprev",
                                     bufs=2)
              nc.vector.memset(mprev_t[0][:], 0.0)

              # conv chains: layer l: src CH[p][l] -> dst CH[p][l+1].
              # All three pairs get dedicated l2 destination tiles so the
              # l2/phase5/bounce trail can shrink to 1 chunk without WAR
              # hazards against l0's lagged xg reads (and xg2 stays intact
              # for the memory phase).
              C3 = [mainp.tile([128, S], f16, name=f"c3_{p}", tag=f"c3_{p}")
                    for p in range(3)]
              CH = [[xg[p], C1[p], C2[p], C3[p]] for p in range(3)]
              FINAL = [CH[p][3] for p in range(3)]

              def emit_sc_tap(t, c):
                  """DVE/Pool tap segment for dst chunk c: cols [max(lag,
                  512c), 512(c+1))."""
                  lo, hi = max(t["lag"], 512 * c), 512 * (c + 1)
                  if lo >= hi:
                      return
                  src, dst = CH[t["p"]][t["l"]], CH[t["p"]][t["l"] + 1]
                  e = nc.gpsimd if t["eng"] == "pool" else nc.vector
                  c_ = t["col"]
                  e.scalar_tensor_tensor(
                      dst[:, lo:hi], src[:, lo - t["lag"]:hi - t["lag"]],
                      csc_sb[:, c_:c_ + 1], dst[:, lo:hi], OP.mult, OP.add)

              def emit_conv(p, l, c, part="all"):
                  """One (pair, layer) chunk: PE-accumulated taps + eviction
                  with bias, then per-chunk DVE tap segments.  part="base"
                  emits only the PE matmuls + eviction; part="segs" only the
                  DVE segments (so l0's eviction can be queued early while
                  its segs sit later in the DVE queue)."""
                  cs_ = slice(512 * c, 512 * (c + 1))
                  src, dst = CH[p][l], CH[p][l + 1]

                  def on_pe(t):
                      return 512 * c >= t["lag"] and c not in t["dve_chunks"]

                  if part in ("all", "base"):
                      ps_c = psp.tile([128, 512], f32, name="psc", tag="B")
                      pe_taps = [t for t in _TAPS
                                 if t["p"] == p and t["l"] == l and on_pe(t)]
                      nc.tensor.matmul(ps_c[:], diag(3 * p + l), src[:, cs_],
                                       start=True, stop=not pe_taps)
                      for i, t in enumerate(pe_taps):
                          a = 512 * c - t["lag"]
                          nc.tensor.matmul(ps_c[:], diag(t["diag"]),
                                           src[:, a:a + 512], start=False,
                                           stop=(i == len(pe_taps) - 1))
                      nc.scalar.activation(
                          dst[:, cs_], ps_c[:], AF.Identity,
                          bias=csc_sb[:, 3 * p + l:3 * p + l + 1], scale=1.0)
                  if part in ("all", "segs"):
                      for t in _TAPS:
                          if t["p"] != p or t["l"] != l or on_pe(t):
                              continue
                          if (c in t["dve_chunks"]
                                  or (t["lag"] % 512 and t["lag"] // 512 == c)):
                              emit_sc_tap(t, c)

              def emit_phase5(c):
                  cs_ = slice(512 * c, 512 * (c + 1))
                  nc.vector.tensor_tensor(FINAL[2][:, cs_], FINAL[2][:, cs_],
                                          mem_t[c][:], OP.add)
                  for p in range(3):
                      ps_h = psp.tile([128, 512], f32, name="psh", tag="B")
                      nc.tensor.matmul(ps_h[:], eind_sb[:, 128 * p:128 * (p + 1)],
                                       hw_t[c][:], start=True, stop=True)
                      nc.vector.tensor_tensor(FINAL[p][:, cs_], FINAL[p][:, cs_],
                                              ps_h[:], OP.mult)

              # per-chunk exchange buffers: [8 dst-core blocks x 384, 64].
              # Core i's block j = its pair features for the 64-token
              # stripe [512c+64j, 512c+64j+64); the AllToAll hands core j
              # that stripe from every core.
              bnc = [dramp.tile([N_CORES * 384, 64], f16, name=f"bin{c}")
                     for c in range(NCK)]
              bnco = [dramp.tile([N_CORES * 384, 64], f16, name=f"bout{c}")
                      for c in range(NCK)]

              def emit_bounce(c):
                  # dst view [384, 8, 64]: (x, j, t) -> dram row 384j+x
                  dv = bnc[c][:].rearrange("(j x) t -> x j t", j=N_CORES)
                  for p in range(3):
                      nc.sync.dma_start(
                          dv[128 * p:128 * (p + 1), :, :],
                          FINAL[p][:, 512 * c:512 * (c + 1)]
                          .rearrange("r (j t) -> r j t", j=N_CORES))
                  nc.gpsimd.collective_compute(
                      "AllToAll", mybir.AluOpType.bypass,
                      replica_groups=[list(range(N_CORES))],
                      ins=[bnc[c][:].opt()], outs=[bnco[c][:].opt()])

              def emit_mix(c):
                  """Mix the core's 64-token stripe of chunk c (4 batches x
                  64 tokens = 256 columns) from the landed piece bnco[c]."""
                  htb = tmpp.tile([128, NB * 256], f16, name="htb", tag="htb",
                                  bufs=1)
                  ht = [htb[:, 256 * i:256 * (i + 1)] for i in range(NB)]
                  gh = [tmpp.tile([128, 256], f16, name=f"ghu{i}",
                                  tag=f"ghu{i}", bufs=1) for i in range(NB)]
                  # per-batch DMA (3-dim AP limit): src rows (2b+g)*384 +
                  # 128p + r -> dst block fb=3g+p, col 64b+t
                  dvw = htb[:].rearrange("r (fb b t) -> r fb b t", fb=NB, b=4)
                  svw = bnco[c][:].rearrange("(b fb r) t -> r b fb t",
                                             b=4, fb=NB)
                  for b in range(4):
                      nc.sync.dma_start(dvw[:, :, b, :], svw[:, b, :, :])
                  for fb in range(NB):
                      ps_pre = psp.tile([128, 256], f32, name="pre", tag="A")
                      for db in range(NB):
                          nc.tensor.matmul(ps_pre[:],
                                           mgT_sb[db][:, 128 * fb:128 * (fb + 1)],
                                           ht[db][:], start=(db == 0),
                                           stop=(db == NB - 1))
                      sg = tmpp.tile([128, 256], f16, name="msig", tag="msig",
                                     bufs=2)
                      nc.scalar.activation(sg[:], ps_pre[:], AF.Sigmoid,
                                           bias=mgb_sb[:, fb:fb + 1], scale=1.0)
                      nc.vector.tensor_tensor(gh[fb][:], ht[fb][:], sg[:],
                                              OP.mult)
                  y_sb = tmpp.tile([128, 2 * HIDDEN], f16, name="ysb",
                                   tag="ysb", bufs=1)
                  for tb in range(2):
                      tr = slice(128 * tb, 128 * (tb + 1))
                      for half in range(2):
                          ps_y = psp.tile([128, 384], f32, name="psy",
                                          tag=("B" if half == 0 else "C"))
                          for fb in range(NB):
                              nc.tensor.matmul(
                                  ps_y[:], gh[fb][:, tr],
                                  mxT_sb[fb][:, 384 * half:384 * (half + 1)],
                                  start=(fb == 0), stop=(fb == NB - 1))
                          yc = HIDDEN * tb + 384 * half
                          nc.vector.tensor_tensor(
                              y_sb[:, yc:yc + 384], ps_y[:],
                              mxb_sb[:, 384 * half:384 * (half + 1)], OP.add)
                      nc.sync.dma_start(
                          y_d[256 * c + 128 * tb:256 * c + 128 * (tb + 1), :],
                          y_sb[:, HIDDEN * tb:HIDDEN * (tb + 1)])

              # Persistent PSUM memory state: ps_M holds the decay-rescaled
              # state M~ = M_0 + sum_j W_j/c_{j+1} (c_h = prod_{j<h} d_j, all
              # per-chunk), accumulated purely by PE matmuls -- the decay
              # multiply is folded into the staged kg (x 1/c_{h+1}) and the
              # per-half Act copy-out applies the c_h rescale.  This removes
              # the per-half DVE M-update entirely, so the serial chain is a
              # PE<->Act ping-pong and never blocks the DVE queue.  Gates
              # are ~0.1-0.16 here so d in [0.84, 0.9] and 1/c_8 <= 4: safe.
              ps_M = psp.tile([64, 256], f32, name="psM", tag="E", bufs=1)

              def emit_memory(ck):
                  x_mem = xg[2]
                  cs_ = slice(512 * ck, 512 * (ck + 1))
                  # --- (1) M-independent precompute ---
                  ps_qa = psp.tile([64, 512], f32, name="psqa", tag="C")
                  nc.tensor.matmul(ps_qa[:], qbd_sb[:, 0:64], x_mem[:, cs_],
                                   start=True, stop=True)
                  q_a = tmpp.tile([64, 512], f16, name="qa", tag="qa", bufs=2)
                  nc.scalar.copy(q_a[:], ps_qa[:])
                  ps_qb = psp.tile([64, 512], f32, name="psqb", tag="C")
                  nc.tensor.matmul(ps_qb[:], qbd_sb[:, 64:128], x_mem[:, cs_],
                                   start=True, stop=True)
                  q_b = tmpp.tile([64, 512], f16, name="qb", tag="qb", bufs=2)
                  nc.scalar.copy(q_b[:], ps_qb[:])
                  kg8 = tmpp.tile([64, 1024], f16, name="kg8", tag="kg8", bufs=2)
                  v8 = tmpp.tile([64, 2048], f16, name="v8", tag="v8", bufs=2)
                  dec8 = tmpp.tile([64, 16], f32, name="dec8", tag="dec8", bufs=2)
                  # decay prefix products, built incrementally (ci[h] only
                  # needs decays through half h, so everything stays in one
                  # loop and ps_kvg is consumed before its buffer recycles):
                  # ch8[2h+hd] = c_h (h=0..8), ci8[2h+hd] = 1/c_{h+1}
                  inv8 = tmpp.tile([64, 16], f32, name="inv8", tag="inv8", bufs=2)
                  ch8 = tmpp.tile([64, 18], f32, name="ch8", tag="ch8", bufs=2)
                  ci8 = tmpp.tile([64, 16], f32, name="ci8", tag="ci8", bufs=2)
                  # [64, 512]: head-a kT blocks in cols 0:256, head-b in
                  # 256:512 (both at base partition 0 for the S matmuls)
                  nc.vector.memset(ch8[:, 0:2], 1.0)
                  for h in range(8):
                      c64 = slice(512 * ck + 64 * h, 512 * ck + 64 * (h + 1))
                      ps_kvg = psp.tile([64, 386], f32, name="pskvg", tag="C")
                      nc.tensor.matmul(ps_kvg[:], x_mem[:, c64], kvg_sb[:],
                                       start=True, stop=True)
                      g_sb = tmpp.tile([64, 2], f32, name="gsb", tag="gsb")
                      for hh in range(2):
                          nc.scalar.activation(g_sb[:, hh:hh + 1],
                                               ps_kvg[:, 384 + hh:385 + hh],
                                               AF.Sigmoid,
                                               bias=gbb_sb[0:64, hh:hh + 1],
                                               scale=1.0)
                      nc.scalar.copy(v8[:, 256 * h:256 * (h + 1)],
                                     ps_kvg[:, 128:384])
                      ps_g = psp.tile([64, 2], f32, name="psg", tag="C")
                      nc.tensor.matmul(ps_g[:], ones_sb[0:64, :], g_sb[:],
                                       start=True, stop=True)
                      nc.scalar.activation(dec8[:, 2 * h:2 * h + 2], ps_g[:],
                                           AF.Identity, bias=1.0, scale=-1.0)
                      nc.vector.reciprocal(inv8[:, 2 * h:2 * h + 2],
                                           dec8[:, 2 * h:2 * h + 2])
                      if h == 0:
                          nc.vector.tensor_copy(ci8[:, 0:2], inv8[:, 0:2])
                      else:
                          nc.vector.tensor_tensor(ch8[:, 2 * h:2 * h + 2],
                                                  ch8[:, 2 * h - 2:2 * h],
                                                  dec8[:, 2 * h - 2:2 * h],
                                                  OP.mult)
                          nc.vector.tensor_tensor(ci8[:, 2 * h:2 * h + 2],
                                                  ci8[:, 2 * h - 2:2 * h],
                                                  inv8[:, 2 * h:2 * h + 2],
                                                  OP.mult)
                      gt = tmpp.tile([64, 2], f32, name="gt", tag="gt")
                      nc.vector.tensor_tensor(gt[:], g_sb[:],
                                              ci8[:, 2 * h:2 * h + 2], OP.mult)
                      for hh in range(2):
                          nc.vector.tensor_scalar(
                              kg8[:, 128 * h + 64 * hh:128 * h + 64 * (hh + 1)],
                              ps_kvg[:, 64 * hh:64 * (hh + 1)],
                              gt[:, hh:hh + 1], None, OP.mult)
                  nc.vector.tensor_tensor(ch8[:, 16:18], ch8[:, 14:16],
                                          dec8[:, 14:16], OP.mult)
                  _mem_stash[ck] = (q_a, q_b, kg8, v8, ch8)

              def emit_memory_serial(ck):
                  q_a, q_b, kg8, v8, ch8 = _mem_stash[ck]
                  # --- (2) serial recurrence: seed ps_M from the previous
                  # chunk's state, then per half: Act copy-out (applying the
                  # c_h rescale), reads-matmul, and W accumulation.
                  mprev = mprev_t[ck]
                  for hd in range(2):
                      nc.tensor.matmul(ps_M[:, 128 * hd:128 * (hd + 1)],
                                       eye64_sb[:], mprev[:, 128 * hd:128 * (hd + 1)],
                                       start=True, stop=True)
                  mnext = tmpp.tile([64, 256], f16, name="mprev", tag="mprev",
                                    bufs=2)
                  mprev_t[ck + 1] = mnext
                  for h in range(8):
                      half = h % 2
                      if half == 0:
                          ps_rd = psp.tile([128, 256], f32, name="psrd", tag="D", bufs=1)
                      if h == 0:
                          m_sb = mprev
                      else:
                          m_sb = tmpp.tile([64, 256], f16, name="msb", tag="msb",
                                           bufs=2)
                          for hd in range(2):
                              nc.scalar.activation(
                                  m_sb[:, 128 * hd:128 * (hd + 1)],
                                  ps_M[:, 128 * hd:128 * (hd + 1)], AF.Identity,
                                  scale=ch8[:, 2 * h + hd:2 * h + hd + 1])
                      nc.tensor.matmul(ps_rd[:, 64 * half:64 * (half + 1)],
                                       m_sb[:, 0:128], q_a[:, 64 * h:64 * (h + 1)],
                                       start=True, stop=True)
                      nc.tensor.matmul(ps_rd[:, 128 + 64 * half:128 + 64 * (half + 1)],
                                       m_sb[:, 128:256], q_b[:, 64 * h:64 * (h + 1)],
                                       start=True, stop=True)
                      nc.tensor.matmul(ps_M[:, 0:128],
                                       kg8[:, 128 * h:128 * h + 64],
                                       v8[:, 256 * h:256 * h + 128],
                                       start=False, stop=True)
                      nc.tensor.matmul(ps_M[:, 128:256],
                                       kg8[:, 128 * h + 64:128 * (h + 1)],
                                       v8[:, 256 * h + 128:256 * (h + 1)],
                                       start=False, stop=True)
                      if half == 1:
                          blk = 4 * ck + h // 2
                          cc = 128 * blk % 512
                          for hh in range(2):
                              nc.scalar.copy(rd_ck[hh][:, cc:cc + 128],
                                             ps_rd[:, 128 * hh:128 * (hh + 1)])
                  for hd in range(2):
                      nc.scalar.activation(
                          mnext[:, 128 * hd:128 * (hd + 1)],
                          ps_M[:, 128 * hd:128 * (hd + 1)], AF.Identity,
                          scale=ch8[:, 16 + hd:17 + hd])
                  ps_o = psp.tile([128, 512], f32, name="pso", tag="C")
                  nc.tensor.matmul(ps_o[:], wot_sb[:, 0:128], rd_ck[0][:],
                                   start=True, stop=False)
                  nc.tensor.matmul(ps_o[:], wot_sb[:, 128:256], rd_ck[1][:],
                                   start=False, stop=True)
                  mem_t[ck] = tmpp.tile([128, 512], f16, name="memo",
                                        tag="memo")
                  nc.scalar.copy(mem_t[ck][:], ps_o[:])

              # ======== main chunk-pipelined driver ========
              # Phase1 runs one chunk AHEAD of everything else so its SwiGLU
              # TTs enter the DVE queue before the previous chunk's l2 seg
              # batch (otherwise the next iteration's PE work -- memory
              # projections, conv l0 -- stalls ~7us per chunk waiting for
              # xg).  memory/l0/l1 at chunk ck; l2 + phase5 + bounce trail
              # by 2 chunks (xg reuse as l2 dst needs l0 lags <= 1024).
              xtt_t = {}

              def load_xt(ck):
                  cs = slice(512 * ck, 512 * (ck + 1))
                  xtt = xtp.tile([128, NB * 512], f16, name="xtt", tag="xtt")
                  nc.sync.dma_start(
                      xtt[:].rearrange("r (i c) -> r i c", i=NB),
                      xT_d[:, cs].rearrange("(i r) c -> r i c", i=NB))
                  xtt_t[ck] = xtt

              def emit_phase1(ck):
                  cs = slice(512 * ck, 512 * (ck + 1))
                  xtt = xtt_t.pop(ck)
                  xt = [xtt[:, 512 * i:512 * (i + 1)] for i in range(NB)]
                  ps_r = psp.tile([8, 512], f32, name="psr", tag="C")
                  for db in range(NB):
                      nc.tensor.matmul(ps_r[:], rT_sb[db][:], xt[db][:],
                                       start=(db == 0), stop=(db == NB - 1))
                  hw_t[ck] = tmpp.tile([8, 512], f16, name="hww", tag="hww",
                                       bufs=4)
                  nc.scalar.activation(hw_t[ck][:], ps_r[:], AF.Sigmoid,
                                       bias=rb_sb[:, 0:1], scale=1.0)
                  for pb in range(3):
                      ps_a = psp.tile([128, 512], f32, name="psa", tag="A")
                      ps_b = psp.tile([128, 512], f32, name="psb", tag="B")
                      for db in range(NB):
                          nc.tensor.matmul(
                              ps_a[:], wg_sb[db][:, 128 * pb:128 * (pb + 1)],
                              xt[db][:], start=(db == 0), stop=(db == NB - 1))
                      for db in range(NB):
                          nc.tensor.matmul(
                              ps_b[:],
                              wg_sb[db][:, 384 + 128 * pb:384 + 128 * (pb + 1)],
                              xt[db][:], start=(db == 0), stop=(db == NB - 1))
                      sig = tmpp.tile([128, 512], f16, name="sig", tag="sig", bufs=2)
                      nc.scalar.activation(sig[:], ps_b[:], AF.Sigmoid)
                      nc.vector.tensor_tensor(xg[pb][:, cs], ps_a[:], sig[:],
                                              OP.mult)

              load_xt(0)
              if _rep == 0:
                  load_p1_weights()
              load_xt(1)
              emit_phase1(0)
              if _rep == 0:
                  load_deferred_consts_a()
              for ck in range(NCK):
                  if ck + 2 < NCK:
                      load_xt(ck + 2)
                  if ck + 1 < NCK:
                      emit_phase1(ck + 1)
                  if _rep == 0 and ck == 0:
                      load_deferred_consts_b()
                  if _rep == 0 and ck == 1:
                      load_mix_weights()
                  if ck < NCK - 1:
                      # ---- memory precompute (M-independent) ----
                      emit_memory(ck)
                      # ---- trailing: l2/phase5/bounce+collective at ck-1
                      # (before l0/l1 so phase5's DVE TTs are not stuck
                      # behind conv segs and the piece's collective starts
                      # early), mixing of the piece landed two iterations
                      # ago ----
                      if ck >= 1:
                          for p in range(3):
                              emit_conv(p, 2, ck - 1)
                          emit_phase5(ck - 1)
                          emit_bounce(ck - 1)
                      # mixes 3+ are deferred to the tail, where the PE
                      # otherwise idles waiting for the last piece's
                      # collective
                      if 2 <= ck <= 4:
                          emit_mix(ck - 2)
                      for p in range(3):
                          emit_conv(p, 0, ck)
                      for p in range(3):
                          emit_conv(p, 1, ck)
                      # ---- memory serial recurrence LAST: its cross-
                      # engine ping-pong then blocks only the tails of the
                      # PE/DVE queues, not ready l2-seg/phase5 bulk work --
                      emit_memory_serial(ck)
                  else:
                      # last iteration: race the chunk-7 chain (memory
                      # serial, convs, bounce) to the collective as early
                      # as possible; the deferred mixes then fill the PE
                      # while piece 7 is in flight.
                      for p in range(3):
                          emit_conv(p, 2, ck - 1)
                      emit_phase5(ck - 1)
                      emit_bounce(ck - 1)
                      emit_memory(ck)
                      emit_memory_serial(ck)
                      for p in range(3):
                          emit_conv(p, 0, ck)
                      for p in range(3):
                          emit_conv(p, 1, ck)
                      for p in range(3):
                          emit_conv(p, 2, ck)
                      emit_phase5(ck)
                      emit_bounce(ck)
                      for c in range(3, NCK):
                          emit_mix(c)

    nc.compile()
    return nc


def _prep_core_inputs(core, inp):
    b, g = core // 2, core % 2
    heads = GROUPS[g]
    f32, f16 = np.float32, np.float16

    x = np.asarray(inp["x"], f32)[b]
    gate_w = np.asarray(inp["gate_w"], f32)
    rows_xg = np.concatenate([np.arange(64 * h, 64 * h + 64) for h in heads])
    W_c = np.concatenate([gate_w[rows_xg], gate_w[768 + rows_xg]], axis=0)

    rT = np.zeros((HIDDEN, 8), f32)
    rT[:, :6] = np.asarray(inp["router_w"], f32)[heads].T
    rb = np.zeros((8, 1), f32)
    rb[:6, 0] = np.asarray(inp["router_b"], f32)[heads]

    conv_w = np.asarray(inp["conv_w"], f32)
    conv_b = np.asarray(inp["conv_b"], f32)
    # conv_sc: cols 0..8 = bias per (p, l); then tap weight columns
    csc = np.zeros((128, _N_COLS), f32)
    # conv_diag: blocks 0..8 = base diag(1 + w3) per (p, l); then PE taps
    cdg = np.zeros((128, 128 * _N_DIAG), f32)
    for p in range(3):
        for l in range(3):
            for hh in range(2):
                head = heads[2 * p + hh]
                rows = slice(64 * hh, 64 * (hh + 1))
                csc[rows, 3 * p + l] = conv_b[head, l, :]
                blk = 3 * p + l
                w3 = 1.0 + conv_w[head, l, :, 3]
                idx = np.arange(64 * hh, 64 * (hh + 1))
                cdg[idx, 128 * blk + idx] = w3
    for t in _TAPS:
        for (gi, hh, k) in t["users"]:
            if gi != g:
                continue
            head = heads[2 * t["p"] + hh]
            w = conv_w[head, t["l"], :, 3 - k]
            idx = np.arange(64 * hh, 64 * (hh + 1))
            if t["diag"] is not None:
                cdg[idx, 128 * t["diag"] + idx] = w
            if t["col"] is not None:
                csc[idx, t["col"]] = w

    ma, mb = heads[4], heads[5]
    ia, ib = MEM_HEADS.index(ma), MEM_HEADS.index(mb)
    Wq = np.asarray(inp["mem_Wq"], f32)
    Wk = np.asarray(inp["mem_Wk"], f32)
    Wv = np.asarray(inp["mem_Wv"], f32)
    Wgw = np.asarray(inp["mem_Wg_w"], f32)
    Wgb = np.asarray(inp["mem_Wg_b"], f32)
    Wo = np.asarray(inp["mem_Wout"], f32)

    qbd = np.zeros((128, 128), f32)
    qbd[0:64, 0:64] = Wq[ia].T
    qbd[64:128, 64:128] = Wq[ib].T
    kvg = np.zeros((128, 386), f32)
    kvg[0:64, 0:64] = Wk[ia].T
    kvg[64:128, 64:128] = Wk[ib].T
    kvg[0:64, 128:256] = Wv[ia].T
    kvg[64:128, 256:384] = Wv[ib].T
    kvg[0:64, 384] = Wgw[ia, 0]
    kvg[64:128, 385] = Wgw[ib, 0]
    gbb = np.zeros((128, 2), f32)
    gbb[:, 0] = Wgb[ia, 0]
    gbb[:, 1] = Wgb[ib, 0]
    wot = np.zeros((128, 256), f32)
    wot[:, 0:64] = Wo[ia].T           # head-a rows 0:64 of stacked out
    wot[:, 128 + 64:256] = Wo[ib].T   # head-b rows 64:128 of stacked out

    eind = np.zeros((8, 384), f32)
    for p in range(3):
        eind[2 * p, 128 * p:128 * p + 64] = 1.0
        eind[2 * p + 1, 128 * p + 64:128 * (p + 1)] = 1.0

    pf = np.concatenate([np.arange(64 * h, 64 * h + 64) for h in PERM_HEADS])
    mixg_w = np.asarray(inp["mixg_w"], f32)
    mix_w = np.asarray(inp["mix_w"], f32)

    return {
        "xT": np.ascontiguousarray(x.T).astype(f16),
        "wgT": np.ascontiguousarray(W_c.T).astype(f16),
        "rT": rT.astype(f16), "rb": rb, "conv_sc": csc,
        "conv_diag": cdg.astype(f16),
        "mem_qbd": qbd.astype(f16), "mem_kvg": kvg.astype(f16),
        "mem_gb_bc": gbb, "mem_WoT": wot.astype(f16),
        "ones64": np.full((128, 64), 1.0 / 64.0, f32),
        "eye64": np.eye(64, dtype=f32).astype(f16),
        "E_ind": eind.astype(f16),
        "mixgT": np.ascontiguousarray(mixg_w[np.ix_(pf, pf)].T).astype(f16),
        "mixgb": np.asarray(inp["mixg_b"], f32)[pf].reshape(HIDDEN, 1).copy(),
        "mixT": np.ascontiguousarray(mix_w[:, pf].T).astype(f16),
        "mixb_bc": np.tile(np.asarray(inp["mix_b"], f32)[None, :], (128, 1)),
    }


def prep_in_maps(inputs):
    return [_prep_core_inputs(c, inputs) for c in range(N_CORES)]


def get_bass():
    if "nc" not in _CACHE:
        _CACHE["nc"] = _build_bass()
    return _CACHE["nc"]


def assemble(results):
    # core j's y rows: 256*c + 64*b + t  ->  out[b, 512*c + 64*j + t]
    out = np.zeros((B, S, HIDDEN), np.float32)
    for j in range(N_CORES):
        y = results[j]["y"].reshape(NCK, B, 64, HIDDEN)
        for c in range(NCK):
            out[:, 512 * c + 64 * j:512 * c + 64 * (j + 1), :] = y[c]
    return out


def kernel(**inputs):
    from concourse import bass_utils
    nc = get_bass()
    in_maps = prep_in_maps(inputs)
    res = bass_utils.run_bass_kernel_spmd(nc, in_maps,
                                          core_ids=list(range(N_CORES)))
    return assemble(res.results)

